# revision 4
# baseline (speedup 1.0000x reference)
"""BertSelfAttention Trainium2 Bass kernel (v2 — pipelined schedule).

Full (unsharded) inputs in, full output out. Internally shards across 8
NeuronCores as (batch b, head-group g): core c handles batch c//2 and
heads [6*(c%2), 6*(c%2)+6) of the 12 heads.

Per-core program (Tile framework), optimized so the ACT (exp) stream
starts ~6us in and never stalls:
  - mask folded into V: probs use exp(0.125*s) only; V rows and the
    ones (denominator) column are pre-scaled by exp(mask_k) -- exactly
    equivalent to adding mask before softmax. This removes the ACT
    bias operand (faster ACTIVATE) and any mask dependency in flash.
  - streaming prologue: mask/bias/Wq0/Wk0 first, hs tiles DMA+transpose
    interleaved with pair-0 Q/K projection chains so the first score
    matmul fires as soon as hs tile 3 lands.
  - V projection chains are emitted inline with pair-0 qc0 flash iters;
    Q/K projections for pairs 1/2 and softmax tails run from a
    background queue pumped between flash iterations (fills PE idle
    slots under the ACT-bound steady state, avoids pair-boundary
    stalls).
  - flash inner loop per (pair, q-chunk of 512): 16 k-tiles:
    scoresT [k,2*512] for both heads of the pair (row-group packed
    K=64 matmuls) -> one ACT exp -> probsT bf16 -> PV accumulates
    ctxT [65, 512] (row 64 = masked softmax denominator).
"""

import os
import sys

sys.path.insert(0, "/opt/trn_rl_repo")

import numpy as np

B, S, D = 4, 2048, 768
H, DH = 12, 64
NCORES = 8
HPC = 6          # heads per core
GSZ = HPC * DH   # 384 output dims per core
P = 128
ND = D // P      # 6 d-tiles
NT = S // P      # 16 k-tiles
QC2 = 512        # q-chunk per flash unit

_cache = {}


def _build(mm_dt_name: str, loop_n: int = 0):
    key = (mm_dt_name, loop_n)
    if key in _cache:
        return _cache[key]

    import concourse.bass as bass
    import concourse.bacc as bacc
    import concourse.mybir as mybir
    from concourse import tile
    from concourse.masks import make_identity

    f32 = mybir.dt.float32
    mm_dt = getattr(mybir.dt, mm_dt_name)
    AF = mybir.ActivationFunctionType

    nc = bacc.Bacc("TRN2", target_bir_lowering=False, debug=False,
                   num_devices=NCORES)

    hs_d = nc.dram_tensor("hs", [S, D], f32, kind="ExternalInput")
    w_d = {p: nc.dram_tensor(f"w{p}", [GSZ, D], f32, kind="ExternalInput")
           for p in "qkv"}
    bias_d = nc.dram_tensor("bias", [3, GSZ], f32, kind="ExternalInput")
    mask_d = nc.dram_tensor("mask", [NT, P], f32, kind="ExternalInput")
    out_d = nc.dram_tensor("out", [S, GSZ], f32, kind="ExternalOutput")

    with tile.TileContext(nc) as tc:
        with tc.tile_pool(name="const", bufs=1) as const_pool, \
             tc.tile_pool(name="persist", bufs=1) as pers:

            ident = const_pool.tile([P, P], f32)
            make_identity(nc, ident[:])

            # ---- persistent SBUF tensors ----
            hsT = pers.tile([P, ND, S], mm_dt, tag="hsT")       # [d%128, dtile, s]
            wT = {p: pers.tile([P, ND, GSZ], mm_dt, tag=f"wT{p}", name=f"wT{p}")
                  for p in "qkv"}
            expmT = pers.tile([P, NT], f32, tag="expmT")        # exp(mask) [k%128, ktile]
            biasT = pers.tile([P, 6], f32, tag="biasT")         # [dim%128, pair*2+proj(q,k)]
            qT = pers.tile([P, 3, S], mm_dt, tag="qT")          # [2*dh, pair, q]
            kT = pers.tile([P, 3, S], mm_dt, tag="kT")
            vsb = pers.tile([P, NT, HPC * (DH + 1)], mm_dt, tag="vsb")
            bvrow = pers.tile([1, GSZ], mm_dt, tag="bvrow")
            onesrow = pers.tile([1, P], mm_dt, tag="onesrow")
            ones6 = pers.tile([P, HPC, 1], mm_dt, tag="ones6")

            nc.vector.memset(onesrow[:], 1.0)
            nc.vector.memset(ones6[:], 1.0)

            import contextlib
            loop_cm = (tc.For_i(0, loop_n, 1,
                                hint_engines=(mybir.EngineType.PE,
                                              mybir.EngineType.Activation,
                                              mybir.EngineType.DVE,
                                              mybir.EngineType.SP))
                       if loop_n else contextlib.nullcontext())
            with loop_cm:
                # one psum pool, three tags (8 banks total):
                #   sa  2x[128,1024]f32 (4 banks) - score tiles + tail transposes
                #   ctx 2x[65,512]f32  (2 banks)  - PV accumulators
                #   m   2x[128,512]f32 (2 banks)  - projections/transposes/misc
                with tc.tile_pool(name="stage", bufs=1) as stage, \
                     tc.tile_pool(name="ps", bufs=2, space="PSUM") as ps, \
                     tc.tile_pool(name="probs", bufs=8) as probs_pool, \
                     tc.tile_pool(name="tailsb", bufs=2) as tailsb, \
                     tc.tile_pool(name="outsb", bufs=4) as outsb:

                    bg = []  # background emission queue (build-time)

                    def pump(n):
                        for _ in range(n):
                            if bg:
                                bg.pop(0)()

                    # ---- mask -> expmT = exp(mask), also warms the ACT table
                    mstage = stage.tile([NT, P], f32, tag="mstage", bufs=1,
                                        name="mstage")
                    nc.sync.dma_start(mstage[:], mask_d[:])
                    mps = ps.tile([P, NT], f32, tag="m", name="mps")
                    nc.tensor.transpose(mps[:], mstage[:], ident[:NT, :NT])
                    nc.scalar.activation(expmT[:], mps[:], AF.Exp)

                    # ---- bias [3, GSZ] -> biasT [dim%128, pair*2+proj]; bv row
                    bstage = stage.tile([3, GSZ], f32, tag="bstage", bufs=1,
                                        name="bstage")
                    nc.sync.dma_start(bstage[:], bias_d[:])
                    for pp in range(3):
                        bps = ps.tile([P, 3], f32, tag="m", name="bps")
                        nc.tensor.transpose(bps[:], bstage[:, pp * P:(pp + 1) * P],
                                            ident[:3, :3])
                        nc.vector.tensor_copy(biasT[:, pp * 2:pp * 2 + 2],
                                              bps[:, 0:2])
                    bvstage = stage.tile([1, GSZ], f32, tag="bvstage", bufs=1,
                                         name="bvstage")
                    nc.sync.dma_start(bvstage[:], bias_d[2:3, :])
                    nc.vector.tensor_copy(bvrow[0:1, :], bvstage[0:1, :])

                    # ---- helpers ----
                    def emit_w_slice(p, r):
                        wstage = stage.tile([P, D], f32, tag="wstage", bufs=3,
                                            name="wstage")
                        nc.sync.dma_start(wstage[:], w_d[p][r * P:(r + 1) * P, :])

                        def half(g):
                            wps = ps.tile([P, 3 * P], f32, tag="m", name="wps")
                            for d3 in range(3):
                                d = g * 3 + d3
                                nc.tensor.transpose(wps[:, d3 * P:(d3 + 1) * P],
                                                    wstage[:, d * P:(d + 1) * P],
                                                    ident[:])
                            nc.vector.tensor_copy(
                                wT[p][:, g * 3:(g + 1) * 3, r * P:(r + 1) * P],
                                wps[:].rearrange("p (d c) -> p d c", c=P))
                        return half

                    def emit_hs_tile(t):
                        hstage = stage.tile([P, D], f32, tag="hstage", bufs=6,
                                            name="hstage")
                        nc.sync.dma_start(hstage[:], hs_d[t * P:(t + 1) * P, :])
                        for g in range(2):
                            hps = ps.tile([P, 3 * P], f32, tag="m", name="hps")
                            for d3 in range(3):
                                d = g * 3 + d3
                                nc.tensor.transpose(hps[:, d3 * P:(d3 + 1) * P],
                                                    hstage[:, d * P:(d + 1) * P],
                                                    ident[:])
                            nc.vector.tensor_copy(
                                hsT[:, g * 3:(g + 1) * 3, t * P:(t + 1) * P],
                                hps[:].rearrange("p (d c) -> p d c", c=P))

                    def emit_qk_chain(pp3, pname, pi, ch):
                        dst = qT if pname == "q" else kT
                        qkp = ps.tile([P, QC2], f32, tag="m", name="qkp")
                        for d in range(ND):
                            nc.tensor.matmul(qkp[:],
                                             wT[pname][:, d, pp3 * P:(pp3 + 1) * P],
                                             hsT[:, d, ch * QC2:(ch + 1) * QC2],
                                             start=(d == 0), stop=(d == ND - 1))
                        nc.vector.tensor_scalar_add(
                            dst[:, pp3, ch * QC2:(ch + 1) * QC2], qkp[:],
                            biasT[:, pp3 * 2 + pi:pp3 * 2 + pi + 1])

                    def emit_v_chain(t):
                        vp = ps.tile([P, GSZ], f32, tag="m", name="vp")
                        for d in range(ND):
                            nc.tensor.matmul(vp[:], hsT[:, d, t * P:(t + 1) * P],
                                             wT["v"][:, d, :],
                                             start=(d == 0), stop=False)
                        nc.tensor.matmul(vp[:], onesrow[0:1, :],
                                         bvrow[0:1, :], start=False, stop=True)
                        # V rows scaled by exp(mask_k) (mask fold)
                        nc.vector.tensor_scalar_mul(
                            vsb[:, t, :].rearrange("p (h c) -> p h c",
                                                   c=DH + 1)[:, :, 0:DH],
                            vp[:].rearrange("p (h c) -> p h c", c=DH),
                            expmT[:, t:t + 1])

                    # ---- wq/wk pair0 (needed first) ----
                    for p in ("q", "k"):
                        h = emit_w_slice(p, 0)
                        h(0)
                        h(1)

                    # ---- denominator ones-columns = exp(mask_k) ----
                    for t in range(NT):
                        nc.vector.tensor_scalar_mul(
                            vsb[:, t, :].rearrange("p (h c) -> p h c",
                                                   c=DH + 1)[:, :, DH:DH + 1],
                            ones6[:], expmT[:, t:t + 1])

                    # ---- hs tiles + pair0 Q/K chains, interleaved; wv after
                    #      the first group so V chains can start early ----
                    for grp in range(4):
                        for t in range(grp * 4, grp * 4 + 4):
                            emit_hs_tile(t)
                        if grp == 0:
                            for r in range(3):
                                h = emit_w_slice("v", r)
                                h(0)
                                h(1)
                        emit_qk_chain(0, "q", 0, grp)
                        emit_qk_chain(0, "k", 1, grp)

                    # ---- tails: ctx->SBUF copies emitted eagerly (they free
                    # the ctx psum slots -- must precede the next qc's PV in
                    # emission order for WAR tracking); the rest goes to bg ----
                    def tail_thunks(pp3, qc, ctxs):
                        st = {}
                        th = []
                        for i in range(2):
                            ctxu = tailsb.tile([DH + 1, QC2], f32,
                                               tag="ctxu", name="ctxu")
                            st[f"cu{i}"] = ctxu
                            nc.vector.tensor_copy(ctxu[:], ctxs[i][:])
                        for s2 in range(QC2 // P):
                            for i in range(2):
                                def tpf(i=i, s2=s2):
                                    if i == 0:
                                        st[f"ot{s2}"] = outsb.tile(
                                            [P, 2 * DH], f32, tag="ot", name="ot")
                                    tp = ps.tile([P, DH + 1], f32, tag="sa",
                                                 name="tp")
                                    nc.tensor.transpose(
                                        tp[:], st[f"cu{i}"][:, s2 * P:(s2 + 1) * P],
                                        ident[:DH + 1, :DH + 1])
                                    rcp = outsb.tile([P, 1], f32, tag="rcp",
                                                     name="rcp")
                                    nc.vector.reciprocal(rcp[:], tp[:, DH:DH + 1])
                                    nc.vector.tensor_scalar_mul(
                                        st[f"ot{s2}"][:, i * DH:(i + 1) * DH],
                                        tp[:, 0:DH], rcp[:])
                                th.append(tpf)

                            def dmaf(s2=s2, pp3=pp3, qc=qc):
                                q0 = qc * QC2 + s2 * P
                                nc.sync.dma_start(
                                    out_d[q0:q0 + P,
                                          pp3 * 2 * DH:(pp3 + 1) * 2 * DH],
                                    st[f"ot{s2}"][:])
                            th.append(dmaf)
                        return th

                    # ---- flash unit: one (pair, q-chunk) ----
                    def emit_flash_qc(pp3, qc, inline_v=False):
                        hA = 2 * pp3
                        ctxs = [ps.tile([DH + 1, QC2], f32, tag="ctx",
                                        name=f"ctx{i}") for i in range(2)]
                        for t in range(NT):
                            sa = ps.tile([P, 2 * QC2], f32, tag="sa", name="sa")
                            for i in range(2):
                                base = i * DH
                                nc.tensor.matmul(
                                    sa[:, i * QC2:(i + 1) * QC2],
                                    kT[base:base + DH, pp3, t * P:(t + 1) * P],
                                    qT[base:base + DH, pp3,
                                       qc * QC2:(qc + 1) * QC2],
                                    start=True, stop=True)
                            pr = probs_pool.tile([P, 2 * QC2], mm_dt,
                                                 tag="pr", name="pr")
                            nc.scalar.activation(pr[:], sa[:], AF.Exp,
                                                 scale=0.125)
                            if inline_v:
                                emit_v_chain(t)
                            for i in range(2):
                                h = hA + i
                                nc.tensor.matmul(
                                    ctxs[i][:],
                                    vsb[:, t, h * (DH + 1):(h + 1) * (DH + 1)],
                                    pr[:, i * QC2:(i + 1) * QC2],
                                    start=(t == 0), stop=(t == NT - 1))
                            if not inline_v:
                                pump(2)
                        bg.extend(tail_thunks(pp3, qc, ctxs))

                    # ---- next-pair W + Q/K projection work (pushed to bg) ----
                    def proj_thunks(pp3):
                        th = []
                        halves = {}
                        for p in ("q", "k"):
                            def dmaw(p=p):
                                halves[p] = emit_w_slice(p, pp3)
                            th.append(dmaw)
                        for p in ("q", "k"):
                            th.append(lambda p=p: halves[p](0))
                            th.append(lambda p=p: halves[p](1))
                        for ch in range(4):
                            for pi, p in enumerate(("q", "k")):
                                th.append(lambda p=p, pi=pi, ch=ch:
                                          emit_qk_chain(pp3, p, pi, ch))
                        return th

                    # ---- main schedule ----
                    for pp3 in range(3):
                        if pp3 > 0:
                            pump(len(bg))  # safety: proj for this pair must be emitted
                        for qc in range(4):
                            emit_flash_qc(pp3, qc,
                                          inline_v=(pp3 == 0 and qc == 0))
                            if pp3 < 2 and qc == 1:
                                bg.extend(proj_thunks(pp3 + 1))
                    pump(len(bg))

    nc.compile()
    _cache[key] = nc
    return nc


def _in_maps(hidden_states, attention_mask, Wq, bq, Wk, bk, Wv, bv):
    maps = []
    for c in range(NCORES):
        b, g = c // 2, c % 2
        sl = slice(g * GSZ, (g + 1) * GSZ)
        maps.append({
            "hs": np.ascontiguousarray(hidden_states[b], dtype=np.float32),
            "wq": np.ascontiguousarray(Wq[sl], dtype=np.float32),
            "wk": np.ascontiguousarray(Wk[sl], dtype=np.float32),
            "wv": np.ascontiguousarray(Wv[sl], dtype=np.float32),
            "bias": np.ascontiguousarray(
                np.stack([bq[sl], bk[sl], bv[sl]]), dtype=np.float32),
            "mask": np.ascontiguousarray(
                attention_mask[b].reshape(NT, P), dtype=np.float32),
        })
    return maps


def kernel(hidden_states, attention_mask, Wq, bq, Wk, bk, Wv, bv,
           _trace=False, _tmpdir=None):
    from concourse.bass_utils import run_bass_kernel_spmd

    nc = _build(os.environ.get("BERT_MM_DT", "bfloat16"))
    maps = _in_maps(np.asarray(hidden_states), np.asarray(attention_mask),
                    np.asarray(Wq), np.asarray(bq), np.asarray(Wk),
                    np.asarray(bk), np.asarray(Wv), np.asarray(bv))
    res = run_bass_kernel_spmd(nc, maps, core_ids=list(range(NCORES)),
                               trace=_trace, tmpdir=_tmpdir)
    out = np.empty((B, S, D), dtype=np.float32)
    for c in range(NCORES):
        b, g = c // 2, c % 2
        out[b, :, g * GSZ:(g + 1) * GSZ] = res.results[c]["out"]
    kernel.last_results = res
    return out


# revision 5
# speedup vs baseline: 1.5062x; 1.5062x over previous
"""BertSelfAttention Trainium2 Bass kernel (v3 — DMA transposes + wavefronts).

Full (unsharded) inputs in, full output out. Internally shards across 8
NeuronCores as (batch b, head-group g): core c handles batch c//2 and
heads [6*(c%2), 6*(c%2)+6) of the 12 heads.

Per-core program (Tile framework). Engine budget per core: ACT exp
stream = 192 x [128,1024] tiles ~= 213us (the roofline); PE ~= 205us;
the schedule keeps both saturated:
  - hs and W reach SBUF pre-transposed without touching PE: gpsimd
    SWDGE DMA casts fp32->bf16 on the fly, then one HWDGE xbar
    dma-transpose per 128-row tile writes [d%128, dtile, s] directly.
  - mask folded into V: probs use exp(0.125*s) only; V rows and the
    ones (denominator) column are pre-scaled by exp(mask_k) -- exactly
    equivalent to adding mask before softmax.
  - wavefront emission: [4 hs tiles | pair-0 Q/K chain | 4 flash iters]
    x4 so the exp stream starts as soon as hs tile 3 lands; V-projection
    chains ride inline with pair-0 qc0; Q/K projections for pairs 1/2
    and softmax tails are pumped from a background queue between flash
    iterations (the Tile scheduler is priority(=emission-order) driven
    and fills PE idle slots with them).
  - softmax tails transpose ctxT via *plain* fp32 matmuls against the
    identity (not transpose-mode) -- PE-HAM counts those as busy, which
    keeps the 2.4GHz clock engaged across chunk boundaries.
  - flash inner loop per (pair, q-chunk of 512): 16 k-tiles:
    scoresT [k,2*512] for both heads (row-group packed K=64 matmuls,
    concurrent) -> one ACT exp -> probsT bf16 -> PV accumulates
    ctxT [65, 512] (row 64 = masked softmax denominator).
"""

import os
import sys

sys.path.insert(0, "/opt/trn_rl_repo")

import numpy as np

B, S, D = 4, 2048, 768
H, DH = 12, 64
NCORES = 8
HPC = 6          # heads per core
GSZ = HPC * DH   # 384 output dims per core
P = 128
ND = D // P      # 6 d-tiles
NT = S // P      # 16 k-tiles
QC2 = 512        # q-chunk per flash unit

_cache = {}


def _build(mm_dt_name: str, loop_n: int = 0):
    key = (mm_dt_name, loop_n)
    if key in _cache:
        return _cache[key]

    import concourse.bass as bass
    import concourse.bacc as bacc
    import concourse.mybir as mybir
    from concourse import tile
    from concourse.masks import make_identity

    f32 = mybir.dt.float32
    mm_dt = getattr(mybir.dt, mm_dt_name)
    AF = mybir.ActivationFunctionType

    nc = bacc.Bacc("TRN2", target_bir_lowering=False, debug=False,
                   num_devices=NCORES)

    hs_d = nc.dram_tensor("hs", [S, D], f32, kind="ExternalInput")
    w_d = {p: nc.dram_tensor(f"w{p}", [GSZ, D], f32, kind="ExternalInput")
           for p in "qkv"}
    bias_d = nc.dram_tensor("bias", [3, GSZ], f32, kind="ExternalInput")
    mask_d = nc.dram_tensor("mask", [NT, P], f32, kind="ExternalInput")
    out_d = nc.dram_tensor("out", [S, GSZ], f32, kind="ExternalOutput")

    with tile.TileContext(nc) as tc:
        with tc.tile_pool(name="const", bufs=1) as const_pool, \
             tc.tile_pool(name="persist", bufs=1) as pers:

            ident = const_pool.tile([P, P], f32)
            make_identity(nc, ident[:])

            # ---- persistent SBUF tensors ----
            hsT = pers.tile([P, ND, S], mm_dt, tag="hsT")       # [d%128, dtile, s]
            wT = {p: pers.tile([P, ND, GSZ], mm_dt, tag=f"wT{p}", name=f"wT{p}")
                  for p in "qkv"}
            expmT = pers.tile([P, NT], f32, tag="expmT")        # exp(mask) [k%128, ktile]
            biasT = pers.tile([P, 6], f32, tag="biasT")         # [dim%128, pair*2+proj(q,k)]
            qT = pers.tile([P, 3, S], mm_dt, tag="qT")          # [2*dh, pair, q]
            kT = pers.tile([P, 3, S], mm_dt, tag="kT")
            vsb = pers.tile([P, NT, HPC * (DH + 1)], mm_dt, tag="vsb")
            bvrow = pers.tile([1, GSZ], mm_dt, tag="bvrow")
            onesrow = pers.tile([1, P], mm_dt, tag="onesrow")
            ones6 = pers.tile([P, HPC, 1], mm_dt, tag="ones6")

            nc.vector.memset(onesrow[:], 1.0)
            nc.vector.memset(ones6[:], 1.0)

            import contextlib
            loop_cm = (tc.For_i(0, loop_n, 1,
                                hint_engines=(mybir.EngineType.PE,
                                              mybir.EngineType.Activation,
                                              mybir.EngineType.DVE,
                                              mybir.EngineType.SP))
                       if loop_n else contextlib.nullcontext())
            with loop_cm:
                # one psum pool, three tags (8 banks total):
                #   sa  2x[128,1024]f32 (4 banks) - score tiles + tail transposes
                #   ctx 2x[65,512]f32  (2 banks)  - PV accumulators
                #   m   2x[128,512]f32 (2 banks)  - projection chains / misc
                with tc.tile_pool(name="stage", bufs=1) as stage, \
                     tc.tile_pool(name="ps", bufs=2, space="PSUM") as ps, \
                     tc.tile_pool(name="probs", bufs=16) as probs_pool, \
                     tc.tile_pool(name="tailsb", bufs=2) as tailsb, \
                     tc.tile_pool(name="outsb", bufs=4) as outsb:

                    bg = []  # background emission queue (build-time)

                    def pump(n):
                        for _ in range(n):
                            if bg:
                                bg.pop(0)()

                    # ---- mask -> expmT = exp(mask), also warms the ACT table
                    mstage = stage.tile([NT, P], f32, tag="mstage", bufs=1,
                                        name="mstage")
                    nc.sync.dma_start(mstage[:], mask_d[:])
                    mps = ps.tile([P, NT], f32, tag="m", name="mps")
                    nc.tensor.transpose(mps[:], mstage[:], ident[:NT, :NT])
                    nc.scalar.activation(expmT[:], mps[:], AF.Exp)

                    # ---- bias [3, GSZ] -> biasT [dim%128, pair*2+proj]; bv row
                    bstage = stage.tile([3, GSZ], f32, tag="bstage", bufs=1,
                                        name="bstage")
                    nc.sync.dma_start(bstage[:], bias_d[:])
                    for pp in range(3):
                        bps = ps.tile([P, 3], f32, tag="m", name="bps")
                        nc.tensor.transpose(bps[:], bstage[:, pp * P:(pp + 1) * P],
                                            ident[:3, :3])
                        nc.vector.tensor_copy(biasT[:, pp * 2:pp * 2 + 2],
                                              bps[:, 0:2])
                    bvstage = stage.tile([1, GSZ], f32, tag="bvstage", bufs=1,
                                         name="bvstage")
                    nc.sync.dma_start(bvstage[:], bias_d[2:3, :])
                    nc.vector.tensor_copy(bvrow[0:1, :], bvstage[0:1, :])

                    # ---- helpers: cast-DMA + xbar transpose-DMA loads ----
                    def emit_w_slice(p, r):
                        wc = stage.tile([P, D], mm_dt, tag="wcast", bufs=3,
                                        name="wc")
                        nc.gpsimd.dma_start(wc[:], w_d[p][r * P:(r + 1) * P, :])
                        nc.sync.dma_start(wT[p][:, :, r * P:(r + 1) * P], wc[:],
                                          transpose=True)

                    def emit_hs_tile(t):
                        hc = stage.tile([P, D], mm_dt, tag="hscast", bufs=6,
                                        name="hc")
                        nc.gpsimd.dma_start(hc[:], hs_d[t * P:(t + 1) * P, :])
                        nc.sync.dma_start(hsT[:, :, t * P:(t + 1) * P], hc[:],
                                          transpose=True)

                    def emit_qk_chain(pp3, pname, pi, ch):
                        dst = qT if pname == "q" else kT
                        qkp = ps.tile([P, QC2], f32, tag="m", name="qkp")
                        for d in range(ND):
                            nc.tensor.matmul(qkp[:],
                                             wT[pname][:, d, pp3 * P:(pp3 + 1) * P],
                                             hsT[:, d, ch * QC2:(ch + 1) * QC2],
                                             start=(d == 0), stop=(d == ND - 1))
                        nc.vector.tensor_scalar_add(
                            dst[:, pp3, ch * QC2:(ch + 1) * QC2], qkp[:],
                            biasT[:, pp3 * 2 + pi:pp3 * 2 + pi + 1])

                    def emit_v_chain(t):
                        vp = ps.tile([P, GSZ], f32, tag="m", name="vp")
                        for d in range(ND):
                            nc.tensor.matmul(vp[:], hsT[:, d, t * P:(t + 1) * P],
                                             wT["v"][:, d, :],
                                             start=(d == 0), stop=False)
                        nc.tensor.matmul(vp[:], onesrow[0:1, :],
                                         bvrow[0:1, :], start=False, stop=True)
                        # V rows scaled by exp(mask_k) (mask fold)
                        nc.vector.tensor_scalar_mul(
                            vsb[:, t, :].rearrange("p (h c) -> p h c",
                                                   c=DH + 1)[:, :, 0:DH],
                            vp[:].rearrange("p (h c) -> p h c", c=DH),
                            expmT[:, t:t + 1])

                    # ---- wq/wk pair0 (needed first), then wv ----
                    for p in ("q", "k"):
                        emit_w_slice(p, 0)

                    # ---- denominator ones-columns = exp(mask_k) ----
                    for t in range(NT):
                        nc.vector.tensor_scalar_mul(
                            vsb[:, t, :].rearrange("p (h c) -> p h c",
                                                   c=DH + 1)[:, :, DH:DH + 1],
                            ones6[:], expmT[:, t:t + 1])

                    # ---- tails: ctx->SBUF copies emitted eagerly (they free
                    # the ctx psum slots); transposes via plain fp32 matmul
                    # against identity (counts as PE-busy for the HAM clock
                    # gate, unlike transpose-mode); rest pumped from bg ----
                    def tail_thunks(pp3, qc, ctxs):
                        st = {}
                        th = []
                        for i in range(2):
                            ctxu = tailsb.tile([DH + 1, QC2], f32,
                                               tag="ctxu", name="ctxu")
                            st[f"cu{i}"] = ctxu
                            nc.vector.tensor_copy(ctxu[:], ctxs[i][:])
                        for s2 in range(QC2 // P):
                            for i in range(2):
                                def tpf(i=i, s2=s2):
                                    if i == 0:
                                        st[f"ot{s2}"] = outsb.tile(
                                            [P, 2 * DH], f32, tag="ot", name="ot")
                                    tp = ps.tile([P, DH + 1], f32, tag="sa",
                                                 name="tp")
                                    nc.tensor.matmul(
                                        tp[:],
                                        st[f"cu{i}"][:, s2 * P:(s2 + 1) * P],
                                        ident[:DH + 1, :DH + 1],
                                        start=True, stop=True)
                                    rcp = outsb.tile([P, 1], f32, tag="rcp",
                                                     name="rcp")
                                    nc.vector.reciprocal(rcp[:], tp[:, DH:DH + 1])
                                    nc.vector.tensor_scalar_mul(
                                        st[f"ot{s2}"][:, i * DH:(i + 1) * DH],
                                        tp[:, 0:DH], rcp[:])
                                th.append(tpf)

                            def dmaf(s2=s2, pp3=pp3, qc=qc):
                                q0 = qc * QC2 + s2 * P
                                nc.sync.dma_start(
                                    out_d[q0:q0 + P,
                                          pp3 * 2 * DH:(pp3 + 1) * 2 * DH],
                                    st[f"ot{s2}"][:])
                            th.append(dmaf)
                        return th

                    # ---- flash iteration (one k-tile of one (pair, qc)) ----
                    def flash_iter(pp3, qc, t, ctxs, inline_v):
                        sa = ps.tile([P, 2 * QC2], f32, tag="sa", name="sa")
                        for i in range(2):
                            base = i * DH
                            nc.tensor.matmul(
                                sa[:, i * QC2:(i + 1) * QC2],
                                kT[base:base + DH, pp3, t * P:(t + 1) * P],
                                qT[base:base + DH, pp3,
                                   qc * QC2:(qc + 1) * QC2],
                                start=True, stop=True)
                        pr = probs_pool.tile([P, 2 * QC2], mm_dt,
                                             tag="pr", name="pr")
                        nc.scalar.activation(pr[:], sa[:], AF.Exp, scale=0.125)
                        if inline_v:
                            emit_v_chain(t)
                        for i in range(2):
                            h = 2 * pp3 + i
                            nc.tensor.matmul(
                                ctxs[i][:],
                                vsb[:, t, h * (DH + 1):(h + 1) * (DH + 1)],
                                pr[:, i * QC2:(i + 1) * QC2],
                                start=(t == 0), stop=(t == NT - 1))
                        if not inline_v:
                            pump(2)

                    def emit_flash_qc(pp3, qc):
                        ctxs = [ps.tile([DH + 1, QC2], f32, tag="ctx",
                                        name=f"ctx{i}") for i in range(2)]
                        for t in range(NT):
                            flash_iter(pp3, qc, t, ctxs, False)
                        bg.extend(tail_thunks(pp3, qc, ctxs))

                    # ---- next-pair W + Q/K projection work (pushed to bg) ----
                    def proj_thunks(pp3):
                        th = []
                        for p in ("q", "k"):
                            th.append(lambda p=p: emit_w_slice(p, pp3))
                        for ch in range(4):
                            for pi, p in enumerate(("q", "k")):
                                th.append(lambda p=p, pi=pi, ch=ch:
                                          emit_qk_chain(pp3, p, pi, ch))
                        return th

                    # ---- pair0 qc0: wavefronts of [hs tiles | chain | flash+V]
                    ctxs0 = [ps.tile([DH + 1, QC2], f32, tag="ctx",
                                     name=f"ctxz{i}") for i in range(2)]
                    for grp in range(4):
                        for t in range(grp * 4, grp * 4 + 4):
                            emit_hs_tile(t)
                        if grp == 0:
                            for r in range(3):
                                emit_w_slice("v", r)
                        emit_qk_chain(0, "q", 0, grp)
                        emit_qk_chain(0, "k", 1, grp)
                        for t in range(grp * 4, grp * 4 + 4):
                            flash_iter(0, 0, t, ctxs0, True)
                    bg.extend(tail_thunks(0, 0, ctxs0))

                    # ---- remaining schedule ----
                    for pp3 in range(3):
                        for qc in range(4):
                            if pp3 == 0 and qc == 0:
                                continue
                            emit_flash_qc(pp3, qc)
                            if pp3 < 2 and qc == 1:
                                bg.extend(proj_thunks(pp3 + 1))
                    pump(len(bg))

    nc.compile()
    _cache[key] = nc
    return nc


def _in_maps(hidden_states, attention_mask, Wq, bq, Wk, bk, Wv, bv):
    maps = []
    for c in range(NCORES):
        b, g = c // 2, c % 2
        sl = slice(g * GSZ, (g + 1) * GSZ)
        maps.append({
            "hs": np.ascontiguousarray(hidden_states[b], dtype=np.float32),
            "wq": np.ascontiguousarray(Wq[sl], dtype=np.float32),
            "wk": np.ascontiguousarray(Wk[sl], dtype=np.float32),
            "wv": np.ascontiguousarray(Wv[sl], dtype=np.float32),
            "bias": np.ascontiguousarray(
                np.stack([bq[sl], bk[sl], bv[sl]]), dtype=np.float32),
            "mask": np.ascontiguousarray(
                attention_mask[b].reshape(NT, P), dtype=np.float32),
        })
    return maps


def kernel(hidden_states, attention_mask, Wq, bq, Wk, bk, Wv, bv,
           _trace=False, _tmpdir=None):
    from concourse.bass_utils import run_bass_kernel_spmd

    nc = _build(os.environ.get("BERT_MM_DT", "bfloat16"))
    maps = _in_maps(np.asarray(hidden_states), np.asarray(attention_mask),
                    np.asarray(Wq), np.asarray(bq), np.asarray(Wk),
                    np.asarray(bk), np.asarray(Wv), np.asarray(bv))
    res = run_bass_kernel_spmd(nc, maps, core_ids=list(range(NCORES)),
                               trace=_trace, tmpdir=_tmpdir)
    out = np.empty((B, S, D), dtype=np.float32)
    for c in range(NCORES):
        b, g = c // 2, c % 2
        out[b, :, g * GSZ:(g + 1) * GSZ] = res.results[c]["out"]
    kernel.last_results = res
    return out
